# revision 8
# baseline (speedup 1.0000x reference)
# Trainium2 Bass kernel for the AttnBlock problem:
#   y = x + proj( attn( groupnorm(x) ) ),  single-head attention over H*W
#   positions, per batch element.  B=4, C=512, H=W=64 (N=4096), f32.
#
# Sharding: 8 cores = 4 batch elements x 2 query-halves.  Each core gets its
# batch's full (C, N) image with the spatial axis rotated so that its 2048
# query positions are local columns [0, 2048).  Attention is invariant to a
# permutation of the key set, and GroupNorm stats are permutation invariant,
# so every core runs an identical (SPMD) program.
#
# All matmuls run in fp8 (e4m3, values kept within TRN's +-240 range) with
# MatmulPerfMode.DoubleRow: each instruction contracts 2x128 partitions at
# 0.5 cycles/row, ~2x the bf16 PE throughput.  Numerical headroom was
# validated against an f64 reference: scale-relative absmax ~5e-3 (vs the
# 2e-2 gate).  Key tricks:
#   - softmax exp is computed as exp(SCALE*s - 2): the constant shift keeps
#     p in [~1e-3, 60] (fp8-safe, softmax-invariant), no max pass needed.
#   - the softmax denominator l = sum_j p is a 16-instruction fp8 ones-matmul
#     on PE (DoubleRow), not a VectorE accumulation chain.
#   - k-bias dropped (softmax-invariant), v-bias folded into the proj bias
#     on the host (bp_eff = bp + wp@bv), q-bias applied during PSUM->SBUF
#     evacuation on VectorE.
#   - x stays resident in SBUF for the final residual add (no re-read).
# Engine balance: PE fp8 matmuls ~82us, ScalarE exp (8.4M elements, the only
# exp-capable engine) ~66us; GroupNorm-apply + k-evac on ScalarE, bn_stats +
# q/v-evac on VectorE; phase 2 is ScalarE-paced with PE PV/proj overlapping
# the next block's exp stream.
import numpy as np
import ml_dtypes

B, C, H, W = 4, 512, 64, 64
N = H * W            # 4096 spatial positions
QH = N // 2          # 2048 queries per core
CH = C // 128        # 4 channel chunks
NJ = N // 128        # 32 key chunks
NI = QH // 512       # 4 query column blocks
EPS = 1e-6
SCALE = float(C) ** -0.5
CEXP = 2.0           # softmax exp shift: p = exp(SCALE*s - CEXP)
NCORES = 8

_CACHE = {}


def _build_module():
    import concourse.bacc as bacc
    import concourse.bass as bass
    import concourse.tile as tile
    from concourse import mybir
    from contextlib import ExitStack

    f32 = mybir.dt.float32
    fp8 = mybir.dt.float8e4
    AF = mybir.ActivationFunctionType
    OP = mybir.AluOpType
    DR = mybir.MatmulPerfMode.DoubleRow

    # Bacc (not plain Bass): its compile() runs generate_event_semaphores /
    # move_matmul_waits_to_ldweights, which enforce the TRN2 one-wait-per-
    # instruction constraint that walrus codegen rejects otherwise.
    nc = bacc.Bacc("TRN2", num_devices=NCORES, enable_asserts=False)

    x_d = nc.dram_tensor("x", [C, N], f32, kind="ExternalInput").ap()
    wqT_d = nc.dram_tensor("wqT", [128, CH, C], fp8, kind="ExternalInput").ap()
    wkT_d = nc.dram_tensor("wkT", [128, CH, C], fp8, kind="ExternalInput").ap()
    wvT_d = nc.dram_tensor("wvT", [128, CH, C], fp8, kind="ExternalInput").ap()
    wpT_d = nc.dram_tensor("wpT", [128, CH, C], fp8, kind="ExternalInput").ap()
    bq_d = nc.dram_tensor("bq", [128, CH], f32, kind="ExternalInput").ap()
    bpe_d = nc.dram_tensor("bpe", [128, CH], f32, kind="ExternalInput").ap()
    gns_d = nc.dram_tensor("gns", [128, CH], f32, kind="ExternalInput").ap()
    gnb_d = nc.dram_tensor("gnb", [128, CH], f32, kind="ExternalInput").ap()
    ind16_d = nc.dram_tensor("ind16", [128, 8], f32, kind="ExternalInput").ap()
    indT_d = nc.dram_tensor("indT", [8, 128], f32, kind="ExternalInput").ap()
    # pair-dim stride must be a multiple of 16 elements for DoubleRow
    # ldweights (s3_lw_dual_fp8_restrictions), hence [128, 2, 16] not [128, 2]
    ones2_d = nc.dram_tensor("ones2", [128, 2, 16], fp8,
                             kind="ExternalInput").ap()
    y_d = nc.dram_tensor("y", [C, QH], f32, kind="ExternalOutput").ap()

    with tile.TileContext(nc) as tc, ExitStack() as ctx:
        consts = ctx.enter_context(tc.tile_pool(name="consts", bufs=1))
        persist = ctx.enter_context(tc.tile_pool(name="persist", bufs=1))

        wpT_sb = consts.tile([128, CH, C], fp8, name="wpT_sb")
        nc.sync.dma_start(wpT_sb, wpT_d)
        bq_sb = consts.tile([128, CH], f32, name="bq_sb")
        nc.sync.dma_start(bq_sb, bq_d)
        bpe_sb = consts.tile([128, CH], f32, name="bpe_sb")
        nc.sync.dma_start(bpe_sb, bpe_d)
        gns_sb = consts.tile([128, CH], f32, name="gns_sb")
        nc.sync.dma_start(gns_sb, gns_d)
        gnb_sb = consts.tile([128, CH], f32, name="gnb_sb")
        nc.sync.dma_start(gnb_sb, gnb_d)
        ind16_sb = consts.tile([128, 8], f32, name="ind16_sb")
        nc.sync.dma_start(ind16_sb, ind16_d)
        indT_sb = consts.tile([8, 128], f32, name="indT_sb")
        nc.sync.dma_start(indT_sb, indT_d)
        ones2_sb = consts.tile([128, 2, 16], fp8, name="ones2_sb")
        nc.sync.dma_start(ones2_sb, ones2_d)
        negc_sb = consts.tile([128, 1], f32, name="negc_sb")
        nc.vector.memset(negc_sb, -CEXP)

        x_all = persist.tile([128, CH, N], f32, name="x_all")
        k_big = persist.tile([128, CH, N], fp8, name="k_big")
        v_big = persist.tile([128, NJ, C], fp8, name="v_big")
        q_big = persist.tile([128, CH, QH], fp8, name="q_big")

        # ---------------- Phase 1: GroupNorm + QKV ----------------
        # 1a computes per-channel GN affine (a, d) from bn_stats; 1b applies
        # the affine slice-by-slice (ScalarE) and feeds fp8 QKV matmuls.
        with tc.tile_pool(name="hp", bufs=3) as hp, \
                tc.tile_pool(name="wts", bufs=1) as wts, \
                tc.tile_pool(name="gt", bufs=2) as gt, \
                tc.tile_pool(name="pqkv", bufs=3, space="PSUM") as pqkv, \
                tc.tile_pool(name="psml", bufs=3, space="PSUM") as psml:

            # 1a: x chunks stream into the resident x_all tile; stats per
            # 512-col slice as the DMA lands; group aggregation per chunk.
            ad_all = gt.tile([128, CH, 2], f32, name="ad_all")
            for cc in range(CH):
                with nc.named_scope(f"gn{cc}"):
                    xv = x_all[:, cc, :].rearrange("p (s f) -> p s f", f=512)
                    stats = gt.tile([128, 8, 6], f32, name="stats")
                    for s in range(8):
                        nc.sync.dma_start(
                            xv[:, s, :],
                            x_d[cc * 128:(cc + 1) * 128,
                                s * 512:(s + 1) * 512])
                        nc.vector.bn_stats(stats[:, s, :], xv[:, s, :])
                    mv = gt.tile([128, 2], f32, name="mv")
                    nc.vector.bn_aggr(mv, stats)
                    # per-channel (mean, mean^2 + var)
                    cm = gt.tile([128, 2], f32, name="cm")
                    nc.vector.tensor_copy(cm[:, 0:1], mv[:, 0:1])
                    nc.vector.scalar_tensor_tensor(
                        out=cm[:, 1:2], in0=mv[:, 0:1], scalar=mv[:, 0:1],
                        in1=mv[:, 1:2], op0=OP.mult, op1=OP.add)
                    # per-chunk group aggregate (16-ch groups sit inside one
                    # chunk) so each chunk's chain overlaps later stats
                    gs_ps = psml.tile([8, 2], f32, name="gs_ps", tag="sm")
                    nc.tensor.matmul(gs_ps, lhsT=ind16_sb, rhs=cm,
                                     start=True, stop=True)
                    gs = gt.tile([8, 2], f32, name="gs")
                    nc.vector.tensor_copy(gs, gs_ps)
                    gv = gt.tile([8, 4], f32, name="gv")
                    nc.vector.scalar_tensor_tensor(
                        out=gv[:, 0:1], in0=gs[:, 0:1], scalar=gs[:, 0:1],
                        in1=gs[:, 1:2], op0=OP.mult, op1=OP.subtract)
                    nc.vector.tensor_scalar(
                        out=gv[:, 0:1], in0=gv[:, 0:1], scalar1=-1.0,
                        scalar2=EPS, op0=OP.mult, op1=OP.add)
                    # rstd = 1/sqrt(var+eps), one Newton refinement
                    nc.scalar.activation(gv[:, 1:2], gv[:, 0:1], AF.Sqrt)
                    nc.vector.reciprocal(gv[:, 2:3], gv[:, 1:2])
                    nc.vector.tensor_mul(gv[:, 3:4], gv[:, 2:3], gv[:, 2:3])
                    nc.vector.tensor_mul(gv[:, 3:4], gv[:, 3:4], gv[:, 0:1])
                    nc.vector.tensor_scalar(
                        out=gv[:, 3:4], in0=gv[:, 3:4], scalar1=-0.5,
                        scalar2=1.5, op0=OP.mult, op1=OP.add)
                    nc.vector.tensor_mul(gs[:, 1:2], gv[:, 2:3], gv[:, 3:4])
                    # broadcast (gmean, rstd) back to channels
                    mr_ps = psml.tile([128, 2], f32, name="mr_ps", tag="sm")
                    nc.tensor.matmul(mr_ps, lhsT=indT_sb, rhs=gs,
                                     start=True, stop=True)
                    ad = ad_all[:, cc, :]
                    nc.vector.tensor_mul(ad[:, 0:1], mr_ps[:, 1:2],
                                         gns_sb[:, cc:cc + 1])
                    nc.vector.tensor_mul(ad[:, 1:2], mr_ps[:, 0:1],
                                         ad[:, 0:1])
                    nc.vector.tensor_sub(ad[:, 1:2], gnb_sb[:, cc:cc + 1],
                                         ad[:, 1:2])
            wqT_sb = wts.tile([128, CH, C], fp8, name="wqT_sb")
            nc.sync.dma_start(wqT_sb, wqT_d)
            wkT_sb = wts.tile([128, CH, C], fp8, name="wkT_sb")
            nc.sync.dma_start(wkT_sb, wkT_d)
            wvT_sb = wts.tile([128, CH, C], fp8, name="wvT_sb")
            nc.sync.dma_start(wvT_sb, wvT_d)

            # 1b: per 512-column slice: GN apply (ScalarE, fp8 out) ->
            # k / q / vT DoubleRow matmuls; PSUM evac on ScalarE (k) and
            # VectorE (q with bias, v).
            for n5 in range(N // 512):
                with nc.named_scope(f"qkv{n5}"):
                    h_sl = hp.tile([128, CH, 512], fp8, name="h_sl")
                    for cc in range(CH):
                        nc.scalar.activation(
                            h_sl[:, cc, :],
                            x_all[:, cc, n5 * 512:(n5 + 1) * 512],
                            AF.Identity,
                            bias=ad_all[:, cc, 1:2], scale=ad_all[:, cc, 0:1])
                    if n5 < NI:  # q for local queries, with bias
                        for oc in range(CH):
                            q_ps = pqkv.tile([128, 512], f32, name="q_ps",
                                             tag="mm")
                            for t in range(CH // 2):
                                nc.tensor.matmul(
                                    q_ps,
                                    lhsT=wqT_sb[:, 2 * t:2 * t + 2,
                                                oc * 128:(oc + 1) * 128],
                                    rhs=h_sl[:, 2 * t:2 * t + 2, :],
                                    start=(t == 0), stop=(t == CH // 2 - 1),
                                    perf_mode=DR)
                            nc.vector.tensor_scalar(
                                out=q_big[:, oc, n5 * 512:(n5 + 1) * 512],
                                in0=q_ps, scalar1=bq_sb[:, oc:oc + 1],
                                scalar2=None, op0=OP.add)
                    for oc in range(CH):  # k, no bias (softmax-invariant)
                        k_ps = pqkv.tile([128, 512], f32, name="k_ps",
                                         tag="mm")
                        for t in range(CH // 2):
                            nc.tensor.matmul(
                                k_ps,
                                lhsT=wkT_sb[:, 2 * t:2 * t + 2,
                                            oc * 128:(oc + 1) * 128],
                                rhs=h_sl[:, 2 * t:2 * t + 2, :],
                                start=(t == 0), stop=(t == CH // 2 - 1),
                                perf_mode=DR)
                        nc.scalar.copy(
                            k_big[:, oc, n5 * 512:(n5 + 1) * 512], k_ps)
                    for j4 in range(4):  # vT (v-bias folded into bp_eff)
                        jn = n5 * 4 + j4
                        v_ps = pqkv.tile([128, 512], f32, name="v_ps",
                                         tag="mm")
                        for t in range(CH // 2):
                            nc.tensor.matmul(
                                v_ps,
                                lhsT=h_sl[:, 2 * t:2 * t + 2,
                                          j4 * 128:(j4 + 1) * 128],
                                rhs=wvT_sb[:, 2 * t:2 * t + 2, :],
                                start=(t == 0), stop=(t == CH // 2 - 1),
                                perf_mode=DR)
                        nc.vector.tensor_copy(v_big[:, jn, :], v_ps)

        # ------------- Phase 2: attention + proj + residual -------------
        # Scores are computed transposed, sT[j,i], so the softmax key-sum is
        # an fp8 ones-matmul on PE and PV contracts j on partitions.  The PE
        # stream per block ic: scores(ic) (ScalarE-exp-paced via 2 PSUM
        # buffers), then l/PV/proj of ic-1, keeping exp(ic) busy while PE
        # works on the previous block's output.
        with tc.tile_pool(name="pp", bufs=2) as pp, \
                tc.tile_pool(name="op", bufs=2) as op_, \
                tc.tile_pool(name="asml", bufs=3) as asml, \
                tc.tile_pool(name="yp", bufs=3) as yp, \
                tc.tile_pool(name="pss", bufs=2, space="PSUM") as pss, \
                tc.tile_pool(name="psl", bufs=2, space="PSUM") as psl, \
                tc.tile_pool(name="pspv", bufs=2, space="PSUM") as pspv, \
                tc.tile_pool(name="drp", bufs=2, space="DRAM") as drp:

            def scores_block(ic):
                # 16 jc-pairs: 4 DR matmuls into a 2-bank PSUM tile, one
                # exp activation over both chunks -> p_big fp8
                p_bl = pp.tile([128, NJ, 512], fp8, name="p_big")
                with nc.named_scope(f"attn{ic}"):
                    for jp in range(NJ // 2):
                        s_ps = pss.tile([128, 2, 512], f32, name="s_ps")
                        for hh in range(2):
                            jc = 2 * jp + hh
                            for t in range(CH // 2):
                                nc.tensor.matmul(
                                    s_ps[:, hh, :],
                                    lhsT=k_big[:, 2 * t:2 * t + 2,
                                               jc * 128:(jc + 1) * 128],
                                    rhs=q_big[:, 2 * t:2 * t + 2,
                                              ic * 512:(ic + 1) * 512],
                                    start=(t == 0), stop=(t == CH // 2 - 1),
                                    perf_mode=DR)
                        nc.scalar.activation(
                            p_bl[:, 2 * jp:2 * jp + 2, :], s_ps, AF.Exp,
                            scale=SCALE, bias=negc_sb)
                return p_bl

            def out_block(ic, p_bl):
                with nc.named_scope(f"out{ic}"):
                    # softmax denominator: fp8 ones-matmul folding all 4096
                    # keys; reciprocal broadcast across partitions via DRAM
                    l_ps = psl.tile([1, 512], f32, name="l_ps")
                    for jp in range(NJ // 2):
                        nc.tensor.matmul(
                            l_ps, lhsT=ones2_sb[:, :, 0:1],
                            rhs=p_bl[:, 2 * jp:2 * jp + 2, :],
                            start=(jp == 0), stop=(jp == NJ // 2 - 1),
                            perf_mode=DR)
                    recip = asml.tile([1, 512], f32, name="recip")
                    nc.vector.reciprocal(recip, l_ps)
                    rd = drp.tile([1, 512], f32, name="rd")
                    nc.sync.dma_start(rd, recip)
                    rb = asml.tile([128, 512], f32, name="rb")
                    rd_b = bass.AP(
                        tensor=rd.tensor, offset=rd.offset,
                        ap=[[0, 128]] + list(rd.ap[1:]))
                    nc.sync.dma_start(rb, rd_b)
                    o_sb = op_.tile([128, CH, 512], fp8, name="o_sb")
                    for cc in range(CH):
                        pv_ps = pspv.tile([128, 512], f32, name="pv_ps",
                                          tag="pv")
                        for jp in range(NJ // 2):
                            nc.tensor.matmul(
                                pv_ps,
                                lhsT=v_big[:, 2 * jp:2 * jp + 2,
                                           cc * 128:(cc + 1) * 128],
                                rhs=p_bl[:, 2 * jp:2 * jp + 2, :],
                                start=(jp == 0), stop=(jp == NJ // 2 - 1),
                                perf_mode=DR)
                        nc.vector.tensor_mul(o_sb[:, cc, :], pv_ps, rb)
                    for oc in range(CH):
                        pj_ps = pspv.tile([128, 512], f32, name="pj_ps",
                                          tag="pv")
                        for t in range(CH // 2):
                            nc.tensor.matmul(
                                pj_ps,
                                lhsT=wpT_sb[:, 2 * t:2 * t + 2,
                                            oc * 128:(oc + 1) * 128],
                                rhs=o_sb[:, 2 * t:2 * t + 2, :],
                                start=(t == 0), stop=(t == CH // 2 - 1),
                                perf_mode=DR)
                        y_sb = yp.tile([128, 512], f32, name="y_sb")
                        nc.vector.scalar_tensor_tensor(
                            out=y_sb, in0=pj_ps,
                            scalar=bpe_sb[:, oc:oc + 1],
                            in1=x_all[:, oc, ic * 512:(ic + 1) * 512],
                            op0=OP.add, op1=OP.add)
                        nc.sync.dma_start(
                            y_d[oc * 128:(oc + 1) * 128,
                                ic * 512:(ic + 1) * 512], y_sb)

            p_prev = scores_block(0)
            for ic in range(1, NI):
                p_cur = scores_block(ic)
                out_block(ic - 1, p_prev)
                p_prev = p_cur
            out_block(NI - 1, p_prev)
    nc.compile()
    return nc


def get_module():
    if "nc" not in _CACHE:
        _CACHE["nc"] = _build_module()
    return _CACHE["nc"]


def _chunked_vec(v):
    # [C] -> [128, CH]: column k holds channels [128k, 128(k+1))
    return np.ascontiguousarray(np.asarray(v, np.float32).reshape(CH, 128).T)


def _wT_chunked_fp8(w):
    # [O, C] weight -> lhsT layout [128, CH, O]: [c_in_chunk, chunk, o]
    wT = np.asarray(w, np.float64).T.reshape(CH, 128, C).transpose(1, 0, 2)
    wT = np.clip(wT, -240.0, 240.0)
    return np.ascontiguousarray(wT.astype(ml_dtypes.float8_e4m3fn))


def make_in_maps(inputs):
    x = np.asarray(inputs["x"], np.float32).reshape(B, C, N)
    ind16 = np.zeros((128, 8), np.float32)
    for c in range(128):
        ind16[c, c // 16] = 1.0 / 16.0
    indT = np.zeros((8, 128), np.float32)
    for c in range(128):
        indT[c // 16, c] = 1.0
    # v-bias folded into an effective proj bias on the host (f64)
    bpe = (np.asarray(inputs["bp"], np.float64)
           + np.asarray(inputs["wp"], np.float64)
           @ np.asarray(inputs["bv"], np.float64))
    shared = {
        "wqT": _wT_chunked_fp8(inputs["wq"]),
        "wkT": _wT_chunked_fp8(inputs["wk"]),
        "wvT": _wT_chunked_fp8(inputs["wv"]),
        "wpT": _wT_chunked_fp8(inputs["wp"]),
        "bq": _chunked_vec(inputs["bq"]),
        "bpe": _chunked_vec(bpe),
        "gns": _chunked_vec(inputs["gn_scale"]),
        "gnb": _chunked_vec(inputs["gn_bias"]),
        "ind16": ind16,
        "indT": indT,
        "ones2": np.ones((128, 2, 16), ml_dtypes.float8_e4m3fn),
    }
    in_maps = []
    for core in range(NCORES):
        b, half = divmod(core, 2)
        xb = x[b]
        if half:
            xl = np.ascontiguousarray(
                np.concatenate([xb[:, QH:], xb[:, :QH]], axis=1))
        else:
            xl = np.ascontiguousarray(xb)
        in_maps.append({"x": xl, **shared})
    return in_maps


def assemble(results, out_dtype=np.float32):
    y = np.empty((B, C, N), np.float32)
    for core in range(NCORES):
        b, half = divmod(core, 2)
        y[b, :, half * QH:(half + 1) * QH] = results[core]["y"]
    return y.reshape(B, C, H, W).astype(out_dtype, copy=False)


def _get_runner():
    """Build the jitted 8-core executable once per process (mirrors
    bass2jax.run_bass_via_pjrt's multi-core branch, without re-tracing
    on every call)."""
    if "runner" in _CACHE:
        return _CACHE["runner"]
    import jax
    from jax.sharding import Mesh, PartitionSpec
    import warnings
    with warnings.catch_warnings():
        warnings.simplefilter("ignore")
        from jax.experimental.shard_map import shard_map
    from concourse import bass2jax, mybir

    nc = get_module()
    bass2jax.install_neuronx_cc_hook()
    partition_name = (nc.partition_id_tensor.name
                      if nc.partition_id_tensor else None)
    in_names, out_names, out_avals = [], [], []
    for alloc in nc.m.functions[0].allocations:
        if not isinstance(alloc, mybir.MemoryLocationSet):
            continue
        name = alloc.memorylocations[0].name
        if alloc.kind == "ExternalInput":
            if name != partition_name:
                in_names.append(name)
        elif alloc.kind == "ExternalOutput":
            out_names.append(name)
            out_avals.append(jax.core.ShapedArray(
                tuple(alloc.tensor_shape), mybir.dt.np(alloc.dtype)))
    all_in_names = list(in_names) + out_names
    if partition_name:
        all_in_names.append(partition_name)

    def _body(*args):
        operands = list(args)
        if partition_name:
            operands.append(bass2jax.partition_id_tensor())
        return tuple(bass2jax._bass_exec_p.bind(
            *operands, out_avals=tuple(out_avals),
            in_names=tuple(all_in_names), out_names=tuple(out_names),
            lowering_input_output_aliases=(),
            sim_require_finite=True, sim_require_nnan=True, nc=nc))

    mesh = Mesh(np.asarray(jax.devices()[:NCORES]), ("core",))
    n_args = len(in_names) + len(out_names)
    fn = jax.jit(shard_map(_body, mesh=mesh,
                           in_specs=(PartitionSpec("core"),) * n_args,
                           out_specs=(PartitionSpec("core"),) * len(out_names),
                           check_rep=False),
                 keep_unused=True)
    zeros = [np.zeros((NCORES * av.shape[0], *av.shape[1:]), av.dtype)
             for av in out_avals]
    _CACHE["runner"] = (fn, in_names, out_names, out_avals, zeros)
    return _CACHE["runner"]


def kernel(**inputs):
    import jax

    fn, in_names, out_names, out_avals, zeros = _get_runner()
    in_maps = make_in_maps(inputs)
    concat = [np.concatenate([np.asarray(in_maps[c][k])
                              for c in range(NCORES)], axis=0)
              for k in in_names]
    outs = fn(*concat, *zeros)
    jax.block_until_ready(outs)
    yi = out_names.index("y")
    y_g = np.asarray(outs[yi]).reshape(NCORES, *out_avals[yi].shape)
    results = [{"y": y_g[c]} for c in range(NCORES)]
    return assemble(results, np.asarray(inputs["x"]).dtype)


if __name__ == "__main__":
    nc = get_module()
    print("module built ok")
